# revision 44
# baseline (speedup 1.0000x reference)
"""Distributed Trainium2 Bass kernel for multi-head attention.

Reference computation (B=4, S=2048, D=1024, H=16 heads, HD=64):
    q = heads(Q @ Wq + bq + Q_lev)
    k = heads(K @ Wk + bk + K_lev)
    v = heads(V @ Wv + bv + V_lev)
    out = softmax(q k^T / sqrt(HD)) v  -> merge heads -> @ Wo + bo

Sharding: 8 cores = 4 batches x 2 head-halves (tensor parallel on the 16
heads: Wq/Wk/Wv split column-wise, Wo row-wise). Each core computes all
2048 queries for its 8 heads and a PARTIAL output [2048, 1024] = ctx_half
@ Wo_half (bf16); the host sums the two partials of each batch (+bo)
during the unshard. No duplicated projection compute and no on-device
collectives.

Device-side layout (feature-major / pre-transposed on the host):
  qT   [HH=512, S]  = Wq_half.T @ Q.T  (+ qlev = (bq + Q_lev).T half)
  kT   [HH, S]      = Wk_half.T @ K.T  (+ klev)
  vaug [tok, 8 heads, 65] = (V @ Wv_half + vlev) with a ones column
                            (row 64 of ctx = softmax denominator)
  scoresT[keys, q] = kT_h.T @ qT_h     (contract over HD=64)
  probsT = exp(scoresT / 8)            (no max subtraction: scores are
                                        N(0,~2) so exp stays < ~1e6)
  ctxT_aug[65, q] = vaug_h.T @ probsT
  ctxT = ctxT_aug[:64] * (1/denominator)  (fast reciprocal + K=2 ones
                                           matmul to broadcast across the
                                           64 head-dim partitions)
  out_partial[q, D] = ctxT.T @ Wo_half

Matmuls run in bf16 (f32 PSUM accumulation). The two K=64 scores matmuls
of a head pair run concurrently in PE row halves (tile_position derived
from base partitions 0/64) and write the two banks of one [128, 1024]
PSUM tile so a single wide ACT exp serves both heads.

Scheduling: ScalarE exp (~1 elem/cycle) and PE matmul streaming are
near-balanced (~285us vs ~275us), so the projections and output
projection are woven into the attention kc loop as PE "fillers" that
execute inside the exp-wait gaps, and the PE stream is software-
pipelined: scores(kc+1) issues before ctx(kc-2). The startup window is
DMA-bound: inputs/weights are fetched with a handful of large strided
DMAs (merged [128, chunk, cols] tiles) ordered by first use across the
three hardware DMA-issue queues (sync/scalar/gpsimd), so the first
scores run ~10us in and exp paces the rest. Only kT[0] n-block 0 and
qT[0] block 0 run before attention call 1; call 1's fillers carry the
rest of kT[0], the whole v projection (vaug[m] lands two kc steps before
ctx needs it) and kT[1]/qT[1]; later calls carry the next chunk's kT/qT
and the previous query block's output projection.
"""

import os
import sys

import numpy as np

for _p in ("/opt/trn_rl_repo", "/root/.axon_site/_ro/trn_rl_repo"):
    if os.path.isdir(_p) and _p not in sys.path:
        sys.path.insert(0, _p)

import ml_dtypes  # noqa: E402

B, S, D, H = 4, 2048, 1024, 16
HD = D // H  # 64
HH = D // 2  # 512 output-feature half per core
NH = H // 2  # 8 heads per core
N_CORES = 8
P = 128  # SBUF partitions
DC = D // P  # 8 chunks of the full (contraction) feature dim
MC = HH // P  # 4 chunks of my output-feature half
KC = S // P  # 16 key chunks
NB = 512  # matmul moving free-dim (one PSUM bank of f32)
NQB = S // NB  # 4 query blocks
CO = 64  # ctx offset inside vaug: [ones, 63 pad, 64 head dims] so the
CW = CO + HD  # denominator lands on PSUM row 0 and ctx on rows 64..127
#              (base-64 spans of 64 partitions are legal APs; a base-32
#              span may only cover 32 partitions. Rows 1..63 are dead.)

_BUILD_CACHE = {}


def _build_nc():
    from concourse import bacc, mybir, tile
    from concourse.bass import _add_dep_helper

    f32 = mybir.dt.float32
    bf16 = mybir.dt.bfloat16
    Exp = mybir.ActivationFunctionType.Exp

    nc = bacc.Bacc("TRN2", target_bir_lowering=False, debug=False, num_devices=N_CORES)

    qt_d = nc.dram_tensor("qt", [D, S], bf16, kind="ExternalInput")
    qlev_d = nc.dram_tensor("qlev", [HH, S], bf16, kind="ExternalInput")
    kt_d = nc.dram_tensor("kt", [D, S], bf16, kind="ExternalInput")
    klev_d = nc.dram_tensor("klev", [HH, S], bf16, kind="ExternalInput")
    vt_d = nc.dram_tensor("vt", [D, S], bf16, kind="ExternalInput")
    vlev_d = nc.dram_tensor("vlev", [S, HH], bf16, kind="ExternalInput")
    wq_d = nc.dram_tensor("wq", [D, HH], bf16, kind="ExternalInput")
    wk_d = nc.dram_tensor("wk", [D, HH], bf16, kind="ExternalInput")
    wv_d = nc.dram_tensor("wv", [D, HH], bf16, kind="ExternalInput")
    wo_d = nc.dram_tensor("wo", [HH, D], bf16, kind="ExternalInput")
    out_d = nc.dram_tensor("out", [S, D], bf16, kind="ExternalOutput")

    # [D, x] dram views as [P, DC, x] (partition-major for merged DMAs)
    qt_v = qt_d.rearrange("(i p) s -> p i s", p=P)
    kt_v = kt_d.rearrange("(i p) s -> p i s", p=P)
    vt_v = vt_d.rearrange("(i p) s -> p i s", p=P)
    wq_v = wq_d.rearrange("(i p) c -> p i c", p=P)
    wk_v = wk_d.rearrange("(i p) c -> p i c", p=P)
    wv_v = wv_d.rearrange("(i p) c -> p i c", p=P)
    wo_v = wo_d.rearrange("(i p) c -> p i c", p=P)

    with tile.TileContext(nc) as tc:
        with (
            tc.tile_pool(name="persist", bufs=1) as persist,
            tc.tile_pool(name="qinp", bufs=2) as qinp,
            tc.tile_pool(name="vinp", bufs=2) as vinp,
            tc.tile_pool(name="lev", bufs=2) as levp,
            tc.tile_pool(name="probs", bufs=4) as prp,
            tc.tile_pool(name="norm", bufs=1) as nrm,
            tc.tile_pool(name="psum", bufs=1, space="PSUM") as psum,
        ):
            # Persistent intermediates (bf16).
            qT = [persist.tile([P, S], bf16, name=f"qT{i}", tag=f"qT{i}") for i in range(MC)]
            kT = [persist.tile([P, S], bf16, name=f"kT{i}", tag=f"kT{i}") for i in range(MC)]
            vaug = [
                persist.tile([P, NH, CW], bf16, name=f"vaug{i}", tag=f"vaug{i}")
                for i in range(KC)
            ]
            ctxT = [persist.tile([P, S], bf16, name=f"ctxT{i}", tag=f"ctxT{i}") for i in range(MC)]
            # ones row at partition 0: broadcasts the per-(head, q)
            # reciprocal (living on PSUM row 0, the vaug ones-column row)
            # across the 64 head-dim partitions via a K=1 matmul.
            ones_t = persist.tile([1, P], bf16, name="ones_t", tag="ones_t")
            # Merged weight/input tiles: one DMA each (DMA issue is ~600ns
            # per instruction on the issuing queue; the startup is gated on
            # instruction count as much as bytes).
            wk_sb = persist.tile([P, DC, HH], bf16, name="wk", tag="wk")
            wq_sb = persist.tile([P, DC, HH], bf16, name="wq", tag="wq")
            wv_sb = persist.tile([P, DC, HH], bf16, name="wv", tag="wv")
            wo_sb = persist.tile([P, MC, D], bf16, name="wo", tag="wo")
            kin = persist.tile([P, DC, S], bf16, name="kin", tag="kin")

            # ---- DMA ordering ----
            # The projection phase is DMA-bound (~12.5MB before attention
            # becomes self-sustaining), so the whole input stream is issued
            # up front on the sync queue in exact first-use order: the
            # descriptor ring back-pressures the queue, so transfers
            # complete roughly in issue order at full bandwidth while the
            # PE trails the stream. The small lev loads ride gpsimd;
            # scalar only runs exp.
            nc.sync.dma_start(kin[:, :, 0:NB], kt_v[:, :, 0:NB])
            nc.sync.dma_start(wk_sb[:, :, 0:P], wk_v[:, :, 0:P])
            qin = {}

            def load_qin(n):
                t = qinp.tile([P, DC, NB], bf16, name="qin", tag="qin")
                nc.sync.dma_start(t[:], qt_v[:, :, n * NB : (n + 1) * NB])
                qin[n] = t

            load_qin(0)
            nc.sync.dma_start(wq_sb[:, :, 0:P], wq_v[:, :, 0:P])
            nc.sync.dma_start(wq_sb[:, :, P:HH], wq_v[:, :, P:HH])
            nc.sync.dma_start(wv_sb[:], wv_v[:])
            nc.sync.dma_start(kin[:, :, NB : 2 * NB], kt_v[:, :, NB : 2 * NB])
            nc.vector.memset(ones_t[:], 1.0)
            vin = {}

            # ---------------- projection fillers -------------
            def kT_chunk_fillers(m, n0=0):
                """kT[m] = Wk[:, m-chunk].T @ K.T: psum groups of 8
                accumulating matmuls + DVE epilogue each."""
                state = {}
                fillers = []
                for n in range(n0, NQB):
                    for kc in range(DC):
                        def mmf(n=n, kc=kc):
                            if kc == 0:
                                state[n] = psum.tile(
                                    [P, NB], f32, name="psk", tag="ps_proj", bufs=2
                                )
                                lev = levp.tile([P, NB], bf16, name="levk", tag="lev")
                                nc.gpsimd.dma_start(
                                    lev[:],
                                    klev_d[m * P : (m + 1) * P, n * NB : (n + 1) * NB],
                                )
                                state["lev", n] = lev
                            nc.tensor.matmul(
                                state[n][:],
                                wk_sb[:, kc, m * P : (m + 1) * P],
                                kin[:, kc, n * NB : (n + 1) * NB],
                                start=(kc == 0),
                                stop=(kc == DC - 1),
                            )
                            if kc == DC - 1:
                                nc.vector.tensor_add(
                                    kT[m][:, n * NB : (n + 1) * NB],
                                    state[n][:],
                                    state["lev", n][:],
                                )
                        fillers.append(mmf)
                return fillers

            def qT_group_fillers(m, n):
                state = {}
                fillers = []
                for kc in range(DC):
                    def mmf(kc=kc):
                        if kc == 0:
                            state[0] = psum.tile(
                                [P, NB], f32, name="psq", tag="ps_proj", bufs=2
                            )
                        nc.tensor.matmul(
                            state[0][:],
                            wq_sb[:, kc, m * P : (m + 1) * P],
                            qin[n][:, kc, :],
                            start=(kc == 0),
                            stop=(kc == DC - 1),
                        )
                        if kc == DC - 1:
                            lev = levp.tile([P, NB], bf16, name="levq", tag="lev")
                            nc.gpsimd.dma_start(
                                lev[:],
                                qlev_d[m * P : (m + 1) * P, n * NB : (n + 1) * NB],
                            )
                            nc.vector.tensor_add(
                                qT[m][:, n * NB : (n + 1) * NB], state[0][:], lev[:]
                            )
                    fillers.append(mmf)
                return fillers

            # v projection: vaug[m] (tokens m*128..) = V @ Wv_half + vlev,
            # head-strided with ones columns. 8 matmuls per chunk.
            vin = {}
            vstate = {}
            vlev_t = {}

            def load_vlev(m):
                t = levp.tile([P, NB], bf16, name="vlev", tag="vlev", bufs=4)
                nc.gpsimd.dma_start(t[:], vlev_d[m * P : (m + 1) * P, :])
                vlev_t[m] = t

            def v_chunk_fillers(m):
                c = m // 4
                fillers = []
                for kc in range(DC):
                    def mmf(kc=kc, m=m, c=c):
                        if kc == 0 and m % 4 == 0:
                            t = vinp.tile([P, DC, NB], bf16, name="vin", tag="vin")
                            nc.sync.dma_start(t[:], vt_v[:, :, c * NB : (c + 1) * NB])
                            vin[c] = t
                        if kc == 0:
                            # vlev prefetched ~3 chunks ahead so the
                            # epilogue add never waits on the transfer
                            if m == 0:
                                for mm_ in range(min(4, KC)):
                                    load_vlev(mm_)
                            elif m + 3 < KC:
                                load_vlev(m + 3)
                        if kc == 0:
                            vstate[0] = psum.tile(
                                [P, NB], f32, name="psv", tag="ps_proj", bufs=2
                            )
                        nc.tensor.matmul(
                            vstate[0][:],
                            vin[c][:, kc, (m % 4) * P : (m % 4 + 1) * P],
                            wv_sb[:, kc, :],
                            start=(kc == 0),
                            stop=(kc == DC - 1),
                        )
                        if kc == DC - 1:
                            nc.vector.tensor_add(
                                vaug[m][:, :, CO:CW],
                                vstate[0][:].rearrange("p (h d) -> p h d", h=NH),
                                vlev_t[m][:].rearrange("p (h d) -> p h d", h=NH),
                            )
                            nc.vector.memset(vaug[m][:, :, 0:CO], 0.0)
                            nc.vector.memset(vaug[m][:, :, 0:1], 1.0)
                    fillers.append(mmf)
                return fillers

            def run_fillers(fillers, k):
                for _ in range(min(k, len(fillers))):
                    fillers.pop(0)()

            last_act = {}

            def emit_attention(qb, hp, fillers=None, per_kc=3, hooks=None, sched=None):
                qs = slice(qb * NB, (qb + 1) * NB)
                fillers = fillers if fillers is not None else []
                hooks = hooks or {}
                cps = [
                    psum.tile([CW, NB], f32, name=f"cps{e}", tag="ctxps", bufs=2)
                    for e in range(2)
                ]
                # software pipeline per kc: scores(kc); exp(kc); PE filler
                # work (projections/outproj) in the exp-wait gap; ctx(kc-2)
                # (lag 2 so ctx never waits on the just-issued exp; probs
                # bufs=4 covers the extra in-flight tile)
                LAG = 2
                prs = {}
                for kc in range(KC + LAG):
                    if kc < KC:
                        sps = psum.tile([P, 2 * NB], f32, name="sps", tag="sps", bufs=2)
                        for e in range(2):
                            rows = slice(e * HD, (e + 1) * HD)
                            # head pair packed in PE row halves
                            nc.tensor.matmul(
                                sps[:, e * NB : (e + 1) * NB],
                                kT[hp][rows, kc * P : (kc + 1) * P],
                                qT[hp][rows, qs],
                                start=True,
                                stop=True,
                            )
                        pr = prp.tile([P, 2 * NB], bf16, name="pr", tag="pr")
                        ai = nc.scalar.activation(pr[:], sps[:], Exp, scale=1.0 / 8.0)
                        last_act["ai"] = ai
                        prs[kc] = pr
                        for h in hooks.get(kc, []):
                            h(ai)
                        run_fillers(fillers, sched[kc] if sched else per_kc)
                    if kc >= LAG:
                        pkc = kc - LAG
                        ppr = prs.pop(pkc)
                        for e in range(2):
                            nc.tensor.matmul(
                                cps[e][:],
                                vaug[pkc][:, 2 * hp + e, :],
                                ppr[:, e * NB : (e + 1) * NB],
                                start=(pkc == 0),
                                stop=(pkc == KC - 1),
                            )
                run_fillers(fillers, len(fillers))
                # 1/denominator straight off PSUM row 0 — no staging copy
                # or DMA; both heads share one tile so a single cast covers
                # them.
                recf = nrm.tile([1, 2 * NB], f32, name="recf", tag="recf", bufs=1)
                recb = nrm.tile([1, 2 * NB], bf16, name="recb", tag="recb", bufs=1)
                for e in range(2):
                    rows = slice(e * HD, (e + 1) * HD)
                    nc.vector.reciprocal_approx_fast(
                        recf[:, e * NB : (e + 1) * NB], cps[e][0:1, :]
                    )
                    # copy unnormalized ctx (normalized in place later)
                    nc.vector.tensor_copy(ctxT[hp][rows, qs], cps[e][CO:CW, :])
                nc.vector.tensor_copy(recb[:], recf[:])
                return (qb, hp, recb)

            def emit_norm_finish(pend):
                # Normalize a head pair (deferred into the next call's
                # filler stream): broadcast each head's reciprocal across
                # its 64 head-dim partitions via a K=1 matmul against the
                # partition-64 ones row.
                qb, hp, recb = pend
                qs = slice(qb * NB, (qb + 1) * NB)
                for e in range(2):
                    rows = slice(e * HD, (e + 1) * HD)
                    bc = psum.tile([P, NB], f32, name="bc", tag="ps_proj", bufs=2)
                    nc.tensor.matmul(
                        bc[:],
                        ones_t[:],
                        recb[:, e * NB : (e + 1) * NB],
                        start=True,
                        stop=True,
                    )
                    nc.vector.tensor_mul(
                        ctxT[hp][rows, qs], ctxT[hp][rows, qs], bc[0:HD, :]
                    )

            def outproj_fillers(qg, n):
                state = {}
                fillers = []
                for dc in range(MC):
                    def mmf(dc=dc):
                        if dc == 0:
                            state[0] = psum.tile(
                                [P, NB], f32, name="pso", tag="ps_proj", bufs=2
                            )
                        nc.tensor.matmul(
                            state[0][:],
                            ctxT[dc][:, qg * P : (qg + 1) * P],
                            wo_sb[:, dc, n * NB : (n + 1) * NB],
                            start=(dc == 0),
                            stop=(dc == MC - 1),
                        )
                        if dc == MC - 1:
                            ot = nrm.tile([P, NB], bf16, name="ot", tag="otile", bufs=4)
                            nc.vector.tensor_copy(ot[:], state[0][:])
                            nc.sync.dma_start(
                                out_d[qg * P : (qg + 1) * P, n * NB : (n + 1) * NB],
                                ot[:],
                            )
                    fillers.append(mmf)
                return fillers

            # ---- interleaved schedule ----
            # Projection-first: the projection phase is DMA-bound anyway, so
            # kT[0], qT[*][qb0] and the whole v projection run before
            # attention, with the PE trailing the input stream at full DMA
            # bandwidth and ScalarE idle. From call 1 on, every attention
            # call is exp-paced (~17.8us) and carries light PE fillers
            # (next kT chunk / next qT groups / previous query block's
            # output projection) in its exp-wait gaps.
            kt0 = kT_chunk_fillers(0)
            for f in kt0[0:DC]:  # n-block 0 (kin-n0 + wk-c0 only)
                f()
            for f in qT_group_fillers(0, 0):
                f()

            # Call 1 (qb0, hp0) carries every remaining projection as PE
            # fillers, in data-arrival order (engine queues are in-order at
            # runtime and the static scheduler does not model DMA latency,
            # so emission order must match the transfer stream). ScalarE
            # starts its exp stream ~15us in. Deadlines: kT[0] n-block j
            # fully emitted at least one step before scores(4j); vaug[m]
            # fully emitted by step m+LAG (the ctx emission point); the
            # lazy vt/kin/wk DMA emissions land on the sync queue between
            # the matching fillers.
            vch = [v_chunk_fillers(m) for m in range(KC)]
            c1 = []
            for m in range(1, MC):
                c1 += qT_group_fillers(m, 0)          # pos 0-23
            c1 += vch[0]                              # pos 24-31 (vt-c0 dma)
            c1 += kt0[DC : 2 * DC]                    # pos 32-39: kT0-n1
            c1.append(lambda: nc.sync.dma_start(
                kin[:, :, 2 * NB : 3 * NB], kt_v[:, :, 2 * NB : 3 * NB]))
            c1 += vch[1] + vch[2] + vch[3] + vch[4]   # pos 41-72 (vt-c1 @ 65)
            c1 += kt0[2 * DC : 3 * DC]                # pos 73-80: kT0-n2
            c1.append(lambda: nc.sync.dma_start(
                kin[:, :, 3 * NB : S], kt_v[:, :, 3 * NB : S]))
            c1 += vch[5] + vch[6] + vch[7]            # pos 82-105
            c1 += kt0[3 * DC :]                       # pos 106-113: kT0-n3
            c1 += vch[8] + vch[9] + vch[10] + vch[11] + vch[12]  # vt-c2/c3
            c1.append(lambda: nc.sync.dma_start(
                wk_sb[:, :, P:HH], wk_v[:, :, P:HH]))
            c1 += vch[13] + vch[14] + vch[15]
            c1 += kT_chunk_fillers(1)                 # call 2's kT chunk

            pend = None
            for qb in range(NQB):
                with nc.named_scope(f"qb{qb}"):
                    for hp in range(MC):
                        fillers = []
                        if qb == 0:
                            if hp == 0:
                                fillers += c1
                            if hp == 2:
                                # wo first used from qb1-hp1; gpsimd queue
                                # keeps it off the startup DMA window.
                                nc.gpsimd.dma_start(wo_sb[:], wo_v[:])
                            if 0 < hp < MC - 1:
                                fillers += kT_chunk_fillers(hp + 1)
                        else:
                            if hp == 0:
                                for m in range(1, MC):
                                    fillers += qT_group_fillers(m, qb)
                            else:
                                # previous qb's outproj: 8 groups over 3 calls
                                og = [(4 * (qb - 1) + g, n) for g in range(4) for n in range(2)]
                                take = {1: og[0:3], 2: og[3:6], 3: og[6:8]}[hp]
                                for qg, n in take:
                                    fillers += outproj_fillers(qg, n)
                        if hp == 1 and qb + 1 < NQB:
                            load_qin(qb + 1)
                        if hp == MC - 1 and qb + 1 < NQB:
                            fillers += qT_group_fillers(0, qb + 1)
                        # Normalize the previous call's head pair as this
                        # call's first filler: keeps the call-boundary PE
                        # chain down to ctx(15)->scores(0) so exp never
                        # waits on the normalize matmul.
                        if pend is not None:
                            prev = pend
                            fillers.insert(0, lambda prev=prev: emit_norm_finish(prev))
                        per_kc = (len(fillers) + KC - 1) // KC
                        pend = emit_attention(qb, hp, fillers, per_kc=max(per_kc, 1))

            def emit_outproj_tail(qg, n, upto=MC, tag="ctxps", use_scalar=False):
                # dc 0..upto-1 into a fresh psum group; rest + epilogue later.
                # Rides the attention's (now idle) PSUM slots; the final
                # normalize's bcast matmul keeps a free ps_proj slot.
                if tag in ("spsA", "spsB"):
                    # ride one half of an (idle) sps slot: two tail groups
                    # share each [128, 1024] scores slot
                    half = 0 if tag == "spsA" else 1
                    ps = psum.tile([P, 2 * NB], f32, name="pso", tag="sps", bufs=2)[
                        :, half * NB : (half + 1) * NB
                    ]
                else:
                    ps = psum.tile([P, NB], f32, name="pso", tag=tag, bufs=2)
                for dc in range(upto):
                    nc.tensor.matmul(
                        ps[:],
                        ctxT[dc][:, qg * P : (qg + 1) * P],
                        wo_sb[:, dc, n * NB : (n + 1) * NB],
                        start=(dc == 0),
                        stop=(dc == MC - 1),
                    )
                def finish():
                    for dc in range(upto, MC):
                        nc.tensor.matmul(
                            ps[:],
                            ctxT[dc][:, qg * P : (qg + 1) * P],
                            wo_sb[:, dc, n * NB : (n + 1) * NB],
                            start=False,
                            stop=(dc == MC - 1),
                        )
                    ot = nrm.tile([P, NB], bf16, name="ot2", tag="otile", bufs=4)
                    if use_scalar:
                        # ScalarE is idle after the final exp; the explicit
                        # dep keeps the scheduler from hoisting these casts
                        # into the exp stream (in-order queue = HOL risk).
                        ci = nc.scalar.copy(ot[:], ps[:])
                        _add_dep_helper(ci.ins, last_act["ai"].ins, sync=True, reason="tail")
                    else:
                        nc.vector.tensor_copy(ot[:], ps[:])
                    nc.sync.dma_start(
                        out_d[qg * P : (qg + 1) * P, n * NB : (n + 1) * NB], ot[:]
                    )
                return finish

            with nc.named_scope("outproj_tail"):
                # last qb's 8 outproj groups; hp 0..2's ctxT chunks are
                # normalized already, so dc 0..2 partials for 5 groups
                # (spread over the now-idle attention PSUM slots) keep the
                # PE at full clock (HAM) through the final normalize's DVE
                # chain; only the dc=3 matmul + epilogue remain per group.
                tail = [(4 * (NQB - 1) + g, n) for g in range(4) for n in range(2)]
                slots = ["ctxps", "ctxps", "spsA", "spsA", "spsB", "spsB", "ps_proj"]
                fins = [
                    emit_outproj_tail(qg, n, upto=MC - 1, tag=tag, use_scalar=(i % 2 == 0))
                    for i, ((qg, n), tag) in enumerate(zip(tail[:7], slots))
                ]
                emit_norm_finish(pend)
                for f in fins:
                    f()
                emit_outproj_tail(*tail[7], use_scalar=True)()

    nc.compile()
    return nc


def get_nc():
    if "nc" not in _BUILD_CACHE:
        _BUILD_CACHE["nc"] = _build_nc()
    return _BUILD_CACHE["nc"]


def make_in_maps(inputs):
    bf16 = ml_dtypes.bfloat16
    f32 = np.float32
    Q = np.asarray(inputs["Q"], f32)
    Q_lev = np.asarray(inputs["Q_lev"], f32)
    K = np.asarray(inputs["K"], f32)
    K_lev = np.asarray(inputs["K_lev"], f32)
    V = np.asarray(inputs["V"], f32)
    V_lev = np.asarray(inputs["V_lev"], f32)
    bq = np.asarray(inputs["bq"], f32)
    bk = np.asarray(inputs["bk"], f32)
    bv = np.asarray(inputs["bv"], f32)
    Wq = np.asarray(inputs["Wq"], f32)
    Wk = np.asarray(inputs["Wk"], f32)
    Wv = np.asarray(inputs["Wv"], f32)
    Wo = np.asarray(inputs["Wo"], f32)

    per_batch = []
    for b in range(B):
        per_batch.append(
            {
                "qt": np.ascontiguousarray(Q[b].T.astype(bf16)),
                "kt": np.ascontiguousarray(K[b].T.astype(bf16)),
                "vt": np.ascontiguousarray(V[b].T.astype(bf16)),
            }
        )
    qlevT = [np.ascontiguousarray((Q_lev[b] + bq).T).astype(bf16) for b in range(B)]
    klevT = [np.ascontiguousarray((K_lev[b] + bk).T).astype(bf16) for b in range(B)]
    vlev = [np.ascontiguousarray(V_lev[b] + bv).astype(bf16) for b in range(B)]

    in_maps = []
    for c in range(N_CORES):
        b, hh = divmod(c, 2)
        fs = slice(hh * HH, (hh + 1) * HH)
        in_maps.append(
            {
                **per_batch[b],
                "qlev": np.ascontiguousarray(qlevT[b][fs]),
                "klev": np.ascontiguousarray(klevT[b][fs]),
                "vlev": np.ascontiguousarray(vlev[b][:, fs]),
                "wq": np.ascontiguousarray(Wq[:, fs].astype(bf16)),
                "wk": np.ascontiguousarray(Wk[:, fs].astype(bf16)),
                "wv": np.ascontiguousarray(Wv[:, fs].astype(bf16)),
                "wo": np.ascontiguousarray(Wo[fs, :].astype(bf16)),
            }
        )
    return in_maps


def combine_outputs(results, inputs):
    bo = np.asarray(inputs["bo"], np.float32)
    out = np.empty((B, S, D), np.float32)
    for b in range(B):
        out[b] = (
            results[2 * b]["out"].astype(np.float32)
            + results[2 * b + 1]["out"].astype(np.float32)
            + bo
        )
    return out


def run_on_cores(inputs, trace=False):
    """Run the SPMD kernel; returns (full_output, BassKernelResults)."""
    from concourse.bass_utils import run_bass_kernel_spmd

    nc = get_nc()
    in_maps = make_in_maps(inputs)
    res = run_bass_kernel_spmd(nc, in_maps, core_ids=list(range(N_CORES)), trace=trace)
    return combine_outputs(res.results, inputs), res


def kernel(**inputs):
    out, _ = run_on_cores(inputs, trace=False)
    return out


if __name__ == "__main__":
    nc = get_nc()
    print("built + compiled OK")


# revision 45
# speedup vs baseline: 1.0307x; 1.0307x over previous
"""Distributed Trainium2 Bass kernel for multi-head attention.

Reference computation (B=4, S=2048, D=1024, H=16 heads, HD=64):
    q = heads(Q @ Wq + bq + Q_lev)
    k = heads(K @ Wk + bk + K_lev)
    v = heads(V @ Wv + bv + V_lev)
    out = softmax(q k^T / sqrt(HD)) v  -> merge heads -> @ Wo + bo

Sharding: 8 cores = 4 batches x 2 head-halves (tensor parallel on the 16
heads: Wq/Wk/Wv split column-wise, Wo row-wise). Each core computes all
2048 queries for its 8 heads and a PARTIAL output [2048, 1024] = ctx_half
@ Wo_half (bf16); the host sums the two partials of each batch (+bo)
during the unshard. No duplicated projection compute and no on-device
collectives.

Device-side layout (feature-major / pre-transposed on the host):
  qT   [HH=512, S]  = Wq_half.T @ Q.T  (+ qlev = (bq + Q_lev).T half)
  kT   [HH, S]      = Wk_half.T @ K.T  (+ klev)
  vaug [tok, 8 heads, 65] = (V @ Wv_half + vlev) with a ones column
                            (row 64 of ctx = softmax denominator)
  scoresT[keys, q] = kT_h.T @ qT_h     (contract over HD=64)
  probsT = exp(scoresT / 8)            (no max subtraction: scores are
                                        N(0,~2) so exp stays < ~1e6)
  ctxT_aug[65, q] = vaug_h.T @ probsT
  ctxT = ctxT_aug[:64] * (1/denominator)  (fast reciprocal + K=2 ones
                                           matmul to broadcast across the
                                           64 head-dim partitions)
  out_partial[q, D] = ctxT.T @ Wo_half

Matmuls run in bf16 (f32 PSUM accumulation). The two K=64 scores matmuls
of a head pair run concurrently in PE row halves (tile_position derived
from base partitions 0/64) and write the two banks of one [128, 1024]
PSUM tile so a single wide ACT exp serves both heads.

Scheduling: ScalarE exp (~1 elem/cycle) and PE matmul streaming are
near-balanced (~285us vs ~275us), so the projections and output
projection are woven into the attention kc loop as PE "fillers" that
execute inside the exp-wait gaps, and the PE stream is software-
pipelined: scores(kc+1) issues before ctx(kc-2). The startup window is
DMA-bound: inputs/weights are fetched with a handful of large strided
DMAs (merged [128, chunk, cols] tiles) ordered by first use across the
three hardware DMA-issue queues (sync/scalar/gpsimd), so the first
scores run ~10us in and exp paces the rest. Only kT[0] n-block 0 and
qT[0] block 0 run before attention call 1; call 1's fillers carry the
rest of kT[0], the whole v projection (vaug[m] lands two kc steps before
ctx needs it) and kT[1]/qT[1]; later calls carry the next chunk's kT/qT
and the previous query block's output projection.
"""

import os
import sys

import numpy as np

for _p in ("/opt/trn_rl_repo", "/root/.axon_site/_ro/trn_rl_repo"):
    if os.path.isdir(_p) and _p not in sys.path:
        sys.path.insert(0, _p)

import ml_dtypes  # noqa: E402

B, S, D, H = 4, 2048, 1024, 16
HD = D // H  # 64
HH = D // 2  # 512 output-feature half per core
NH = H // 2  # 8 heads per core
N_CORES = 8
P = 128  # SBUF partitions
DC = D // P  # 8 chunks of the full (contraction) feature dim
MC = HH // P  # 4 chunks of my output-feature half
KC = S // P  # 16 key chunks
NB = 512  # matmul moving free-dim (one PSUM bank of f32)
NQB = S // NB  # 4 query blocks
CO = 64  # ctx offset inside vaug: [ones, 63 pad, 64 head dims] so the
CW = CO + HD  # denominator lands on PSUM row 0 and ctx on rows 64..127
#              (base-64 spans of 64 partitions are legal APs; a base-32
#              span may only cover 32 partitions. Rows 1..63 are dead.)

_BUILD_CACHE = {}


def _build_nc():
    from concourse import bacc, mybir, tile
    from concourse.bass import _add_dep_helper

    f32 = mybir.dt.float32
    bf16 = mybir.dt.bfloat16
    Exp = mybir.ActivationFunctionType.Exp

    nc = bacc.Bacc("TRN2", target_bir_lowering=False, debug=False, num_devices=N_CORES)

    qt_d = nc.dram_tensor("qt", [D, S], bf16, kind="ExternalInput")
    qlev_d = nc.dram_tensor("qlev", [HH, S], bf16, kind="ExternalInput")
    kt_d = nc.dram_tensor("kt", [D, S], bf16, kind="ExternalInput")
    klev_d = nc.dram_tensor("klev", [HH, S], bf16, kind="ExternalInput")
    vt_d = nc.dram_tensor("vt", [D, S], bf16, kind="ExternalInput")
    vlev_d = nc.dram_tensor("vlev", [S, HH], bf16, kind="ExternalInput")
    wq_d = nc.dram_tensor("wq", [D, HH], bf16, kind="ExternalInput")
    wk_d = nc.dram_tensor("wk", [D, HH], bf16, kind="ExternalInput")
    wv_d = nc.dram_tensor("wv", [D, HH], bf16, kind="ExternalInput")
    wo_d = nc.dram_tensor("wo", [HH, D], bf16, kind="ExternalInput")
    out_d = nc.dram_tensor("out", [S, D], bf16, kind="ExternalOutput")

    # [D, x] dram views as [P, DC, x] (partition-major for merged DMAs)
    qt_v = qt_d.rearrange("(i p) s -> p i s", p=P)
    kt_v = kt_d.rearrange("(i p) s -> p i s", p=P)
    vt_v = vt_d.rearrange("(i p) s -> p i s", p=P)
    wq_v = wq_d.rearrange("(i p) c -> p i c", p=P)
    wk_v = wk_d.rearrange("(i p) c -> p i c", p=P)
    wv_v = wv_d.rearrange("(i p) c -> p i c", p=P)
    wo_v = wo_d.rearrange("(i p) c -> p i c", p=P)

    with tile.TileContext(nc) as tc:
        with (
            tc.tile_pool(name="persist", bufs=1) as persist,
            tc.tile_pool(name="qinp", bufs=2) as qinp,
            tc.tile_pool(name="vinp", bufs=2) as vinp,
            tc.tile_pool(name="lev", bufs=2) as levp,
            tc.tile_pool(name="probs", bufs=4) as prp,
            tc.tile_pool(name="norm", bufs=1) as nrm,
            tc.tile_pool(name="psum", bufs=1, space="PSUM") as psum,
        ):
            # Persistent intermediates (bf16).
            qT = [persist.tile([P, S], bf16, name=f"qT{i}", tag=f"qT{i}") for i in range(MC)]
            kT = [persist.tile([P, S], bf16, name=f"kT{i}", tag=f"kT{i}") for i in range(MC)]
            vaug = [
                persist.tile([P, NH, CW], bf16, name=f"vaug{i}", tag=f"vaug{i}")
                for i in range(KC)
            ]
            ctxT = [persist.tile([P, S], bf16, name=f"ctxT{i}", tag=f"ctxT{i}") for i in range(MC)]
            # ones row at partition 0: broadcasts the per-(head, q)
            # reciprocal (living on PSUM row 0, the vaug ones-column row)
            # across the 64 head-dim partitions via a K=1 matmul.
            ones_t = persist.tile([1, P], bf16, name="ones_t", tag="ones_t")
            # Merged weight/input tiles: one DMA each (DMA issue is ~600ns
            # per instruction on the issuing queue; the startup is gated on
            # instruction count as much as bytes).
            wk_sb = persist.tile([P, DC, HH], bf16, name="wk", tag="wk")
            wq_sb = persist.tile([P, DC, HH], bf16, name="wq", tag="wq")
            wv_sb = persist.tile([P, DC, HH], bf16, name="wv", tag="wv")
            wo_sb = persist.tile([P, MC, D], bf16, name="wo", tag="wo")
            kin = persist.tile([P, DC, S], bf16, name="kin", tag="kin")

            # ---- DMA ordering ----
            # The projection phase is DMA-bound (~12.5MB before attention
            # becomes self-sustaining), so the whole input stream is issued
            # up front on the sync queue in exact first-use order: the
            # descriptor ring back-pressures the queue, so transfers
            # complete roughly in issue order at full bandwidth while the
            # PE trails the stream. The small lev loads ride gpsimd;
            # scalar only runs exp.
            nc.sync.dma_start(kin[:, :, 0:NB], kt_v[:, :, 0:NB])
            nc.sync.dma_start(wk_sb[:, :, 0:P], wk_v[:, :, 0:P])
            qin = {}

            def load_qin(n):
                t = qinp.tile([P, DC, NB], bf16, name="qin", tag="qin")
                nc.sync.dma_start(t[:], qt_v[:, :, n * NB : (n + 1) * NB])
                qin[n] = t

            load_qin(0)
            nc.sync.dma_start(wq_sb[:, :, 0:P], wq_v[:, :, 0:P])
            nc.sync.dma_start(wq_sb[:, :, P:HH], wq_v[:, :, P:HH])
            nc.sync.dma_start(wv_sb[:], wv_v[:])
            nc.sync.dma_start(kin[:, :, NB : 2 * NB], kt_v[:, :, NB : 2 * NB])
            nc.vector.memset(ones_t[:], 1.0)
            vin = {}

            # ---------------- projection fillers -------------
            def kT_chunk_fillers(m, n0=0):
                """kT[m] = Wk[:, m-chunk].T @ K.T: psum groups of 8
                accumulating matmuls + DVE epilogue each."""
                state = {}
                fillers = []
                for n in range(n0, NQB):
                    for kc in range(DC):
                        def mmf(n=n, kc=kc):
                            if kc == 0:
                                state[n] = psum.tile(
                                    [P, NB], f32, name="psk", tag="ps_proj", bufs=2
                                )
                                lev = levp.tile([P, NB], bf16, name="levk", tag="lev")
                                nc.gpsimd.dma_start(
                                    lev[:],
                                    klev_d[m * P : (m + 1) * P, n * NB : (n + 1) * NB],
                                )
                                state["lev", n] = lev
                            nc.tensor.matmul(
                                state[n][:],
                                wk_sb[:, kc, m * P : (m + 1) * P],
                                kin[:, kc, n * NB : (n + 1) * NB],
                                start=(kc == 0),
                                stop=(kc == DC - 1),
                            )
                            if kc == DC - 1:
                                nc.vector.tensor_add(
                                    kT[m][:, n * NB : (n + 1) * NB],
                                    state[n][:],
                                    state["lev", n][:],
                                )
                        fillers.append(mmf)
                return fillers

            def qT_group_fillers(m, n):
                state = {}
                fillers = []
                for kc in range(DC):
                    def mmf(kc=kc):
                        if kc == 0:
                            state[0] = psum.tile(
                                [P, NB], f32, name="psq", tag="ps_proj", bufs=2
                            )
                        nc.tensor.matmul(
                            state[0][:],
                            wq_sb[:, kc, m * P : (m + 1) * P],
                            qin[n][:, kc, :],
                            start=(kc == 0),
                            stop=(kc == DC - 1),
                        )
                        if kc == DC - 1:
                            lev = levp.tile([P, NB], bf16, name="levq", tag="lev")
                            nc.gpsimd.dma_start(
                                lev[:],
                                qlev_d[m * P : (m + 1) * P, n * NB : (n + 1) * NB],
                            )
                            nc.vector.tensor_add(
                                qT[m][:, n * NB : (n + 1) * NB], state[0][:], lev[:]
                            )
                    fillers.append(mmf)
                return fillers

            # v projection: vaug[m] (tokens m*128..) = V @ Wv_half + vlev,
            # head-strided with ones columns. 8 matmuls per chunk.
            vin = {}
            vstate = {}
            vlev_t = {}

            def load_vlev(m):
                t = levp.tile([P, NB], bf16, name="vlev", tag="vlev", bufs=4)
                nc.gpsimd.dma_start(t[:], vlev_d[m * P : (m + 1) * P, :])
                vlev_t[m] = t

            def v_chunk_fillers(m):
                c = m // 4
                fillers = []
                for kc in range(DC):
                    def mmf(kc=kc, m=m, c=c):
                        if kc == 0 and m % 4 == 0:
                            t = vinp.tile([P, DC, NB], bf16, name="vin", tag="vin")
                            nc.sync.dma_start(t[:], vt_v[:, :, c * NB : (c + 1) * NB])
                            vin[c] = t
                        if kc == 0:
                            # vlev prefetched ~3 chunks ahead so the
                            # epilogue add never waits on the transfer
                            if m == 0:
                                for mm_ in range(min(4, KC)):
                                    load_vlev(mm_)
                            elif m + 3 < KC:
                                load_vlev(m + 3)
                        if kc == 0:
                            vstate[0] = psum.tile(
                                [P, NB], f32, name="psv", tag="ps_proj", bufs=2
                            )
                        nc.tensor.matmul(
                            vstate[0][:],
                            vin[c][:, kc, (m % 4) * P : (m % 4 + 1) * P],
                            wv_sb[:, kc, :],
                            start=(kc == 0),
                            stop=(kc == DC - 1),
                        )
                        if kc == DC - 1:
                            nc.vector.tensor_add(
                                vaug[m][:, :, CO:CW],
                                vstate[0][:].rearrange("p (h d) -> p h d", h=NH),
                                vlev_t[m][:].rearrange("p (h d) -> p h d", h=NH),
                            )
                            nc.vector.memset(vaug[m][:, :, 0:CO], 0.0)
                            nc.vector.memset(vaug[m][:, :, 0:1], 1.0)
                    fillers.append(mmf)
                return fillers

            def run_fillers(fillers, k):
                for _ in range(min(k, len(fillers))):
                    fillers.pop(0)()

            last_act = {}

            def emit_attention(qb, hp, fillers=None, per_kc=3, hooks=None, sched=None):
                qs = slice(qb * NB, (qb + 1) * NB)
                fillers = fillers if fillers is not None else []
                hooks = hooks or {}
                cps = [
                    psum.tile([CW, NB], f32, name=f"cps{e}", tag="ctxps", bufs=2)
                    for e in range(2)
                ]
                # software pipeline per kc: scores(kc); exp(kc); PE filler
                # work (projections/outproj) in the exp-wait gap; ctx(kc-2)
                # (lag 2 so ctx never waits on the just-issued exp; probs
                # bufs=4 covers the extra in-flight tile)
                LAG = 2
                prs = {}
                for kc in range(KC + LAG):
                    if kc < KC:
                        sps = psum.tile([P, 2 * NB], f32, name="sps", tag="sps", bufs=2)
                        for e in range(2):
                            rows = slice(e * HD, (e + 1) * HD)
                            # head pair packed in PE row halves
                            nc.tensor.matmul(
                                sps[:, e * NB : (e + 1) * NB],
                                kT[hp][rows, kc * P : (kc + 1) * P],
                                qT[hp][rows, qs],
                                start=True,
                                stop=True,
                            )
                        pr = prp.tile([P, 2 * NB], bf16, name="pr", tag="pr")
                        ai = nc.scalar.activation(pr[:], sps[:], Exp, scale=1.0 / 8.0)
                        last_act["ai"] = ai
                        prs[kc] = pr
                        for h in hooks.get(kc, []):
                            h(ai)
                        run_fillers(fillers, sched[kc] if sched else per_kc)
                    if kc >= LAG:
                        pkc = kc - LAG
                        ppr = prs.pop(pkc)
                        for e in range(2):
                            nc.tensor.matmul(
                                cps[e][:],
                                vaug[pkc][:, 2 * hp + e, :],
                                ppr[:, e * NB : (e + 1) * NB],
                                start=(pkc == 0),
                                stop=(pkc == KC - 1),
                            )
                run_fillers(fillers, len(fillers))
                recbs = []
                for e in range(2):
                    rows = slice(e * HD, (e + 1) * HD)
                    # 1/denominator straight off PSUM row 0 — no staging
                    # copy or DMA.
                    recf = nrm.tile([1, NB], f32, name=f"recf{e}", tag=f"recf{e}", bufs=1)
                    nc.vector.reciprocal_approx_fast(recf[:], cps[e][0:1, :])
                    recb = nrm.tile([1, NB], bf16, name=f"recb{e}", tag=f"recb{e}", bufs=1)
                    nc.vector.tensor_copy(recb[:], recf[:])
                    recbs.append(recb)
                    # copy unnormalized ctx (normalized in place later)
                    nc.vector.tensor_copy(ctxT[hp][rows, qs], cps[e][CO:CW, :])
                return (qb, hp, recbs)

            def emit_norm_finish(pend):
                # Normalize a head pair (deferred into the next call's
                # filler stream): broadcast each head's reciprocal across
                # its 64 head-dim partitions via a K=1 matmul against the
                # partition-64 ones row.
                qb, hp, recbs = pend
                qs = slice(qb * NB, (qb + 1) * NB)
                for e in range(2):
                    rows = slice(e * HD, (e + 1) * HD)
                    bc = psum.tile([P, NB], f32, name="bc", tag="ps_proj", bufs=2)
                    nc.tensor.matmul(
                        bc[:], ones_t[:], recbs[e][:], start=True, stop=True
                    )
                    nc.vector.tensor_mul(
                        ctxT[hp][rows, qs], ctxT[hp][rows, qs], bc[0:HD, :]
                    )

            def outproj_fillers(qg, n):
                state = {}
                fillers = []
                for dc in range(MC):
                    def mmf(dc=dc):
                        if dc == 0:
                            state[0] = psum.tile(
                                [P, NB], f32, name="pso", tag="ps_proj", bufs=2
                            )
                        nc.tensor.matmul(
                            state[0][:],
                            ctxT[dc][:, qg * P : (qg + 1) * P],
                            wo_sb[:, dc, n * NB : (n + 1) * NB],
                            start=(dc == 0),
                            stop=(dc == MC - 1),
                        )
                        if dc == MC - 1:
                            ot = nrm.tile([P, NB], bf16, name="ot", tag="otile", bufs=4)
                            nc.vector.tensor_copy(ot[:], state[0][:])
                            nc.sync.dma_start(
                                out_d[qg * P : (qg + 1) * P, n * NB : (n + 1) * NB],
                                ot[:],
                            )
                    fillers.append(mmf)
                return fillers

            # ---- interleaved schedule ----
            # Projection-first: the projection phase is DMA-bound anyway, so
            # kT[0], qT[*][qb0] and the whole v projection run before
            # attention, with the PE trailing the input stream at full DMA
            # bandwidth and ScalarE idle. From call 1 on, every attention
            # call is exp-paced (~17.8us) and carries light PE fillers
            # (next kT chunk / next qT groups / previous query block's
            # output projection) in its exp-wait gaps.
            kt0 = kT_chunk_fillers(0)
            for f in kt0[0:DC]:  # n-block 0 (kin-n0 + wk-c0 only)
                f()
            for f in qT_group_fillers(0, 0):
                f()

            # Call 1 (qb0, hp0) carries every remaining projection as PE
            # fillers, in data-arrival order (engine queues are in-order at
            # runtime and the static scheduler does not model DMA latency,
            # so emission order must match the transfer stream). ScalarE
            # starts its exp stream ~15us in. Deadlines: kT[0] n-block j
            # fully emitted at least one step before scores(4j); vaug[m]
            # fully emitted by step m+LAG (the ctx emission point); the
            # lazy vt/kin/wk DMA emissions land on the sync queue between
            # the matching fillers.
            vch = [v_chunk_fillers(m) for m in range(KC)]
            c1 = []
            for m in range(1, MC):
                c1 += qT_group_fillers(m, 0)          # pos 0-23
            c1 += vch[0]                              # pos 24-31 (vt-c0 dma)
            c1 += kt0[DC : 2 * DC]                    # pos 32-39: kT0-n1
            c1.append(lambda: nc.sync.dma_start(
                kin[:, :, 2 * NB : 3 * NB], kt_v[:, :, 2 * NB : 3 * NB]))
            c1 += vch[1] + vch[2] + vch[3] + vch[4]   # pos 41-72 (vt-c1 @ 65)
            c1 += kt0[2 * DC : 3 * DC]                # pos 73-80: kT0-n2
            c1.append(lambda: nc.sync.dma_start(
                kin[:, :, 3 * NB : S], kt_v[:, :, 3 * NB : S]))
            c1 += vch[5] + vch[6] + vch[7]            # pos 82-105
            c1 += kt0[3 * DC :]                       # pos 106-113: kT0-n3
            c1 += vch[8] + vch[9] + vch[10] + vch[11] + vch[12]  # vt-c2/c3
            c1.append(lambda: nc.sync.dma_start(
                wk_sb[:, :, P:HH], wk_v[:, :, P:HH]))
            c1 += vch[13] + vch[14] + vch[15]
            c1 += kT_chunk_fillers(1)                 # call 2's kT chunk

            pend = None
            for qb in range(NQB):
                with nc.named_scope(f"qb{qb}"):
                    for hp in range(MC):
                        fillers = []
                        if qb == 0:
                            if hp == 0:
                                fillers += c1
                            if hp == 2:
                                # wo first used from qb1-hp1; gpsimd queue
                                # keeps it off the startup DMA window.
                                nc.gpsimd.dma_start(wo_sb[:], wo_v[:])
                            if 0 < hp < MC - 1:
                                fillers += kT_chunk_fillers(hp + 1)
                        else:
                            if hp == 0:
                                for m in range(1, MC):
                                    fillers += qT_group_fillers(m, qb)
                            else:
                                # previous qb's outproj: 8 groups over 3 calls
                                og = [(4 * (qb - 1) + g, n) for g in range(4) for n in range(2)]
                                take = {1: og[0:3], 2: og[3:6], 3: og[6:8]}[hp]
                                for qg, n in take:
                                    fillers += outproj_fillers(qg, n)
                        if hp == 1 and qb + 1 < NQB:
                            load_qin(qb + 1)
                        if hp == MC - 1 and qb + 1 < NQB:
                            fillers += qT_group_fillers(0, qb + 1)
                        # Normalize the previous call's head pair as this
                        # call's first filler: keeps the call-boundary PE
                        # chain down to ctx(15)->scores(0) so exp never
                        # waits on the normalize matmul.
                        if pend is not None:
                            prev = pend
                            fillers.insert(0, lambda prev=prev: emit_norm_finish(prev))
                        per_kc = (len(fillers) + KC - 1) // KC
                        pend = emit_attention(qb, hp, fillers, per_kc=max(per_kc, 1))

            def emit_outproj_tail(qg, n, upto=MC, tag="ctxps", use_scalar=False):
                # dc 0..upto-1 into a fresh psum group; rest + epilogue later.
                # Rides the attention's (now idle) PSUM slots; the final
                # normalize's bcast matmul keeps a free ps_proj slot.
                if tag in ("spsA", "spsB"):
                    # ride one half of an (idle) sps slot: two tail groups
                    # share each [128, 1024] scores slot
                    half = 0 if tag == "spsA" else 1
                    ps = psum.tile([P, 2 * NB], f32, name="pso", tag="sps", bufs=2)[
                        :, half * NB : (half + 1) * NB
                    ]
                else:
                    ps = psum.tile([P, NB], f32, name="pso", tag=tag, bufs=2)
                for dc in range(upto):
                    nc.tensor.matmul(
                        ps[:],
                        ctxT[dc][:, qg * P : (qg + 1) * P],
                        wo_sb[:, dc, n * NB : (n + 1) * NB],
                        start=(dc == 0),
                        stop=(dc == MC - 1),
                    )
                def finish():
                    for dc in range(upto, MC):
                        nc.tensor.matmul(
                            ps[:],
                            ctxT[dc][:, qg * P : (qg + 1) * P],
                            wo_sb[:, dc, n * NB : (n + 1) * NB],
                            start=False,
                            stop=(dc == MC - 1),
                        )
                    ot = nrm.tile([P, NB], bf16, name="ot2", tag="otile", bufs=4)
                    if use_scalar:
                        # ScalarE is idle after the final exp; the explicit
                        # dep keeps the scheduler from hoisting these casts
                        # into the exp stream (in-order queue = HOL risk).
                        ci = nc.scalar.copy(ot[:], ps[:])
                        _add_dep_helper(ci.ins, last_act["ai"].ins, sync=True, reason="tail")
                    else:
                        nc.vector.tensor_copy(ot[:], ps[:])
                    nc.sync.dma_start(
                        out_d[qg * P : (qg + 1) * P, n * NB : (n + 1) * NB], ot[:]
                    )
                return finish

            with nc.named_scope("outproj_tail"):
                # last qb's 8 outproj groups; hp 0..2's ctxT chunks are
                # normalized already, so dc 0..2 partials for 5 groups
                # (spread over the now-idle attention PSUM slots) keep the
                # PE at full clock (HAM) through the final normalize's DVE
                # chain; only the dc=3 matmul + epilogue remain per group.
                tail = [(4 * (NQB - 1) + g, n) for g in range(4) for n in range(2)]
                slots = ["ctxps", "ctxps", "spsA", "spsA", "spsB", "spsB", "ps_proj"]
                fins = [
                    emit_outproj_tail(qg, n, upto=MC - 1, tag=tag, use_scalar=(i % 2 == 0))
                    for i, ((qg, n), tag) in enumerate(zip(tail[:7], slots))
                ]
                emit_norm_finish(pend)
                for f in fins:
                    f()
                emit_outproj_tail(*tail[7], use_scalar=True)()

    nc.compile()
    return nc


def get_nc():
    if "nc" not in _BUILD_CACHE:
        _BUILD_CACHE["nc"] = _build_nc()
    return _BUILD_CACHE["nc"]


def make_in_maps(inputs):
    bf16 = ml_dtypes.bfloat16
    f32 = np.float32
    Q = np.asarray(inputs["Q"], f32)
    Q_lev = np.asarray(inputs["Q_lev"], f32)
    K = np.asarray(inputs["K"], f32)
    K_lev = np.asarray(inputs["K_lev"], f32)
    V = np.asarray(inputs["V"], f32)
    V_lev = np.asarray(inputs["V_lev"], f32)
    bq = np.asarray(inputs["bq"], f32)
    bk = np.asarray(inputs["bk"], f32)
    bv = np.asarray(inputs["bv"], f32)
    Wq = np.asarray(inputs["Wq"], f32)
    Wk = np.asarray(inputs["Wk"], f32)
    Wv = np.asarray(inputs["Wv"], f32)
    Wo = np.asarray(inputs["Wo"], f32)

    per_batch = []
    for b in range(B):
        per_batch.append(
            {
                "qt": np.ascontiguousarray(Q[b].T.astype(bf16)),
                "kt": np.ascontiguousarray(K[b].T.astype(bf16)),
                "vt": np.ascontiguousarray(V[b].T.astype(bf16)),
            }
        )
    qlevT = [np.ascontiguousarray((Q_lev[b] + bq).T).astype(bf16) for b in range(B)]
    klevT = [np.ascontiguousarray((K_lev[b] + bk).T).astype(bf16) for b in range(B)]
    vlev = [np.ascontiguousarray(V_lev[b] + bv).astype(bf16) for b in range(B)]

    in_maps = []
    for c in range(N_CORES):
        b, hh = divmod(c, 2)
        fs = slice(hh * HH, (hh + 1) * HH)
        in_maps.append(
            {
                **per_batch[b],
                "qlev": np.ascontiguousarray(qlevT[b][fs]),
                "klev": np.ascontiguousarray(klevT[b][fs]),
                "vlev": np.ascontiguousarray(vlev[b][:, fs]),
                "wq": np.ascontiguousarray(Wq[:, fs].astype(bf16)),
                "wk": np.ascontiguousarray(Wk[:, fs].astype(bf16)),
                "wv": np.ascontiguousarray(Wv[:, fs].astype(bf16)),
                "wo": np.ascontiguousarray(Wo[fs, :].astype(bf16)),
            }
        )
    return in_maps


def combine_outputs(results, inputs):
    bo = np.asarray(inputs["bo"], np.float32)
    out = np.empty((B, S, D), np.float32)
    for b in range(B):
        out[b] = (
            results[2 * b]["out"].astype(np.float32)
            + results[2 * b + 1]["out"].astype(np.float32)
            + bo
        )
    return out


def run_on_cores(inputs, trace=False):
    """Run the SPMD kernel; returns (full_output, BassKernelResults)."""
    from concourse.bass_utils import run_bass_kernel_spmd

    nc = get_nc()
    in_maps = make_in_maps(inputs)
    res = run_bass_kernel_spmd(nc, in_maps, core_ids=list(range(N_CORES)), trace=trace)
    return combine_outputs(res.results, inputs), res


def kernel(**inputs):
    out, _ = run_on_cores(inputs, trace=False)
    return out


if __name__ == "__main__":
    nc = get_nc()
    print("built + compiled OK")


# revision 46
# speedup vs baseline: 1.0340x; 1.0032x over previous
"""Distributed Trainium2 Bass kernel for multi-head attention.

Reference computation (B=4, S=2048, D=1024, H=16 heads, HD=64):
    q = heads(Q @ Wq + bq + Q_lev)
    k = heads(K @ Wk + bk + K_lev)
    v = heads(V @ Wv + bv + V_lev)
    out = softmax(q k^T / sqrt(HD)) v  -> merge heads -> @ Wo + bo

Sharding: 8 cores = 4 batches x 2 head-halves (tensor parallel on the 16
heads: Wq/Wk/Wv split column-wise, Wo row-wise). Each core computes all
2048 queries for its 8 heads and a PARTIAL output [2048, 1024] = ctx_half
@ Wo_half (bf16); the host sums the two partials of each batch (+bo)
during the unshard. No duplicated projection compute (the query-split
alternative recomputes the K/V projections on both cores of a pair,
+17% PE work) and no on-device collectives.

Device-side layout (feature-major / pre-transposed on the host):
  qT   [HH=512, S]  = Wq_half.T @ Q.T  (+ qlev = (bq + Q_lev).T half)
  kT   [HH, S]      = Wk_half.T @ K.T  (+ klev)
  vaug [tok, 8 heads, 128] = [ones, 63 dead, V @ Wv_half + vlev] so the
        ctx matmul emits the softmax denominator on PSUM row 0 and ctx on
        rows 64..127 (AP partition bases are limited to 0/32/64/96, and a
        base-32 access may span at most 32 partitions)
  scoresT[keys, q] = kT_h.T @ qT_h     (contract over HD=64)
  probsT = exp(scoresT / 8)            (no max subtraction: scores are
                                        N(0,~2) so exp stays < ~1e6)
  ctx_aug[128, q] = vaug_h.T @ probsT
  ctxT = ctx_aug[64:128] * (1/denominator)  (DVE fast reciprocal read
        straight off PSUM row 0, bf16 cast, then a K=1 matmul against a
        ones row broadcasts it across the 64 head-dim partitions)
  out_partial[q, D] = ctxT.T @ Wo_half

Matmuls run in bf16 (f32 PSUM accumulation). The two K=64 scores matmuls
of a head pair run concurrently in PE row halves (tile_position derived
from base partitions 0/64) and write the two banks of one [128, 1024]
PSUM tile so a single wide ACT exp serves both heads.

Engine balance per core: ScalarE exp is 256 ACTIVATEs x ~1.11us =
~285us (the hard floor; exp is ScalarE-only at 1 elem/cycle/lane) and
PE streaming is ~273us, so the schedule keeps ScalarE saturated and
hides everything else in its gaps:
 - scores(kc)/exp(kc)/ctx(kc-2) software pipeline per key chunk; all
   projections and the output projection are "fillers" run between
   scores and ctx inside each of the 16 attention calls.
 - call 1 carries the whole v projection + kT[0] n1-3 + kT[1] + the
   remaining qT(qb0) groups, ordered by DATA ARRIVAL: engine queues are
   in-order at runtime and the static tile scheduler does not model DMA
   latency, so a filler emitted before its input lands head-of-line
   blocks every later scores -> the filler order must match the DMA
   stream order, and kT[0] n-block j must be fully emitted a step before
   scores(4j) (same-queue waits would deadlock otherwise).
 - the input stream (12.5MB before attention self-sustains) is issued up
   front on the sync queue in exact first-use order; the descriptor ring
   back-pressures the queue so transfers complete roughly in issue order
   at full bandwidth (~330GB/s). In-flight DMAs from different queues
   fair-share the 16 engines, so the lev sideloads ride gpsimd and
   scalar issues nothing (an issue there would delay exp).
 - each call's normalize runs as the NEXT call's first filler so the
   call-boundary PE chain is just ctx(15)->scores(0).
 - the tail (last query block's outproj) pre-accumulates dc 0..2 for 7
   groups into the idle attention PSUM slots (keeping HAM at full clock)
   before the final normalize, leaving one matmul + epilogue per group;
   epilogue casts alternate DVE/ScalarE, the ScalarE ones carrying an
   explicit dep on the final exp so the scheduler cannot hoist them into
   the exp stream.
"""

import os
import sys

import numpy as np

for _p in ("/opt/trn_rl_repo", "/root/.axon_site/_ro/trn_rl_repo"):
    if os.path.isdir(_p) and _p not in sys.path:
        sys.path.insert(0, _p)

import ml_dtypes  # noqa: E402

B, S, D, H = 4, 2048, 1024, 16
HD = D // H  # 64
HH = D // 2  # 512 output-feature half per core
NH = H // 2  # 8 heads per core
N_CORES = 8
P = 128  # SBUF partitions
DC = D // P  # 8 chunks of the full (contraction) feature dim
MC = HH // P  # 4 chunks of my output-feature half
KC = S // P  # 16 key chunks
NB = 512  # matmul moving free-dim (one PSUM bank of f32)
NQB = S // NB  # 4 query blocks
CO = 64  # ctx offset inside vaug: [ones, 63 pad, 64 head dims] so the
CW = CO + HD  # denominator lands on PSUM row 0 and ctx on rows 64..127
#              (base-64 spans of 64 partitions are legal APs; a base-32
#              span may only cover 32 partitions. Rows 1..63 are dead.)

_BUILD_CACHE = {}


def _build_nc():
    from concourse import bacc, mybir, tile
    from concourse.bass import _add_dep_helper

    f32 = mybir.dt.float32
    bf16 = mybir.dt.bfloat16
    Exp = mybir.ActivationFunctionType.Exp

    nc = bacc.Bacc("TRN2", target_bir_lowering=False, debug=False, num_devices=N_CORES)

    qt_d = nc.dram_tensor("qt", [D, S], bf16, kind="ExternalInput")
    qlev_d = nc.dram_tensor("qlev", [HH, S], bf16, kind="ExternalInput")
    kt_d = nc.dram_tensor("kt", [D, S], bf16, kind="ExternalInput")
    klev_d = nc.dram_tensor("klev", [HH, S], bf16, kind="ExternalInput")
    vt_d = nc.dram_tensor("vt", [D, S], bf16, kind="ExternalInput")
    vlev_d = nc.dram_tensor("vlev", [S, HH], bf16, kind="ExternalInput")
    wq_d = nc.dram_tensor("wq", [D, HH], bf16, kind="ExternalInput")
    wk_d = nc.dram_tensor("wk", [D, HH], bf16, kind="ExternalInput")
    wv_d = nc.dram_tensor("wv", [D, HH], bf16, kind="ExternalInput")
    wo_d = nc.dram_tensor("wo", [HH, D], bf16, kind="ExternalInput")
    out_d = nc.dram_tensor("out", [S, D], bf16, kind="ExternalOutput")

    # [D, x] dram views as [P, DC, x] (partition-major for merged DMAs)
    qt_v = qt_d.rearrange("(i p) s -> p i s", p=P)
    kt_v = kt_d.rearrange("(i p) s -> p i s", p=P)
    vt_v = vt_d.rearrange("(i p) s -> p i s", p=P)
    wq_v = wq_d.rearrange("(i p) c -> p i c", p=P)
    wk_v = wk_d.rearrange("(i p) c -> p i c", p=P)
    wv_v = wv_d.rearrange("(i p) c -> p i c", p=P)
    wo_v = wo_d.rearrange("(i p) c -> p i c", p=P)

    with tile.TileContext(nc) as tc:
        with (
            tc.tile_pool(name="persist", bufs=1) as persist,
            tc.tile_pool(name="qinp", bufs=2) as qinp,
            tc.tile_pool(name="vinp", bufs=2) as vinp,
            tc.tile_pool(name="lev", bufs=2) as levp,
            tc.tile_pool(name="probs", bufs=4) as prp,
            tc.tile_pool(name="norm", bufs=1) as nrm,
            tc.tile_pool(name="psum", bufs=1, space="PSUM") as psum,
        ):
            # Persistent intermediates (bf16).
            qT = [persist.tile([P, S], bf16, name=f"qT{i}", tag=f"qT{i}") for i in range(MC)]
            kT = [persist.tile([P, S], bf16, name=f"kT{i}", tag=f"kT{i}") for i in range(MC)]
            vaug = [
                persist.tile([P, NH, CW], bf16, name=f"vaug{i}", tag=f"vaug{i}")
                for i in range(KC)
            ]
            ctxT = [persist.tile([P, S], bf16, name=f"ctxT{i}", tag=f"ctxT{i}") for i in range(MC)]
            # ones row at partition 0: broadcasts the per-(head, q)
            # reciprocal (living on PSUM row 0, the vaug ones-column row)
            # across the 64 head-dim partitions via a K=1 matmul.
            ones_t = persist.tile([1, P], bf16, name="ones_t", tag="ones_t")
            # Merged weight/input tiles: one DMA each (DMA issue is ~600ns
            # per instruction on the issuing queue; the startup is gated on
            # instruction count as much as bytes).
            wk_sb = persist.tile([P, DC, HH], bf16, name="wk", tag="wk")
            wq_sb = persist.tile([P, DC, HH], bf16, name="wq", tag="wq")
            wv_sb = persist.tile([P, DC, HH], bf16, name="wv", tag="wv")
            wo_sb = persist.tile([P, MC, D], bf16, name="wo", tag="wo")
            kin = persist.tile([P, DC, S], bf16, name="kin", tag="kin")

            # ---- DMA ordering ----
            # The projection phase is DMA-bound (~12.5MB before attention
            # becomes self-sustaining), so the whole input stream is issued
            # up front on the sync queue in exact first-use order: the
            # descriptor ring back-pressures the queue, so transfers
            # complete roughly in issue order at full bandwidth while the
            # PE trails the stream. The small lev loads ride gpsimd;
            # scalar only runs exp.
            nc.sync.dma_start(kin[:, :, 0:NB], kt_v[:, :, 0:NB])
            nc.sync.dma_start(wk_sb[:, :, 0:P], wk_v[:, :, 0:P])
            qin = {}

            def load_qin(n):
                t = qinp.tile([P, DC, NB], bf16, name="qin", tag="qin")
                nc.sync.dma_start(t[:], qt_v[:, :, n * NB : (n + 1) * NB])
                qin[n] = t

            load_qin(0)
            nc.sync.dma_start(wq_sb[:, :, 0:P], wq_v[:, :, 0:P])
            nc.sync.dma_start(wq_sb[:, :, P:HH], wq_v[:, :, P:HH])
            nc.sync.dma_start(wv_sb[:], wv_v[:])
            nc.sync.dma_start(kin[:, :, NB : 2 * NB], kt_v[:, :, NB : 2 * NB])
            nc.vector.memset(ones_t[:], 1.0)
            vin = {}

            # ---------------- projection fillers -------------
            def kT_chunk_fillers(m, n0=0):
                """kT[m] = Wk[:, m-chunk].T @ K.T: psum groups of 8
                accumulating matmuls + DVE epilogue each."""
                state = {}
                fillers = []
                for n in range(n0, NQB):
                    for kc in range(DC):
                        def mmf(n=n, kc=kc):
                            if kc == 0:
                                state[n] = psum.tile(
                                    [P, NB], f32, name="psk", tag="ps_proj", bufs=2
                                )
                                lev = levp.tile([P, NB], bf16, name="levk", tag="lev")
                                nc.gpsimd.dma_start(
                                    lev[:],
                                    klev_d[m * P : (m + 1) * P, n * NB : (n + 1) * NB],
                                )
                                state["lev", n] = lev
                            nc.tensor.matmul(
                                state[n][:],
                                wk_sb[:, kc, m * P : (m + 1) * P],
                                kin[:, kc, n * NB : (n + 1) * NB],
                                start=(kc == 0),
                                stop=(kc == DC - 1),
                            )
                            if kc == DC - 1:
                                nc.vector.tensor_add(
                                    kT[m][:, n * NB : (n + 1) * NB],
                                    state[n][:],
                                    state["lev", n][:],
                                )
                        fillers.append(mmf)
                return fillers

            def qT_group_fillers(m, n):
                state = {}
                fillers = []
                for kc in range(DC):
                    def mmf(kc=kc):
                        if kc == 0:
                            state[0] = psum.tile(
                                [P, NB], f32, name="psq", tag="ps_proj", bufs=2
                            )
                        nc.tensor.matmul(
                            state[0][:],
                            wq_sb[:, kc, m * P : (m + 1) * P],
                            qin[n][:, kc, :],
                            start=(kc == 0),
                            stop=(kc == DC - 1),
                        )
                        if kc == DC - 1:
                            lev = levp.tile([P, NB], bf16, name="levq", tag="lev")
                            nc.gpsimd.dma_start(
                                lev[:],
                                qlev_d[m * P : (m + 1) * P, n * NB : (n + 1) * NB],
                            )
                            nc.vector.tensor_add(
                                qT[m][:, n * NB : (n + 1) * NB], state[0][:], lev[:]
                            )
                    fillers.append(mmf)
                return fillers

            # v projection: vaug[m] (tokens m*128..) = V @ Wv_half + vlev,
            # head-strided with ones columns. 8 matmuls per chunk.
            vin = {}
            vstate = {}
            vlev_t = {}

            def load_vlev(m):
                t = levp.tile([P, NB], bf16, name="vlev", tag="vlev", bufs=4)
                nc.gpsimd.dma_start(t[:], vlev_d[m * P : (m + 1) * P, :])
                vlev_t[m] = t

            def v_chunk_fillers(m):
                c = m // 4
                fillers = []
                for kc in range(DC):
                    def mmf(kc=kc, m=m, c=c):
                        if kc == 0 and m % 4 == 0:
                            t = vinp.tile([P, DC, NB], bf16, name="vin", tag="vin")
                            nc.sync.dma_start(t[:], vt_v[:, :, c * NB : (c + 1) * NB])
                            vin[c] = t
                        if kc == 0:
                            # vlev prefetched ~3 chunks ahead so the
                            # epilogue add never waits on the transfer
                            if m == 0:
                                for mm_ in range(min(4, KC)):
                                    load_vlev(mm_)
                            elif m + 3 < KC:
                                load_vlev(m + 3)
                        if kc == 0:
                            vstate[0] = psum.tile(
                                [P, NB], f32, name="psv", tag="ps_proj", bufs=2
                            )
                        nc.tensor.matmul(
                            vstate[0][:],
                            vin[c][:, kc, (m % 4) * P : (m % 4 + 1) * P],
                            wv_sb[:, kc, :],
                            start=(kc == 0),
                            stop=(kc == DC - 1),
                        )
                        if kc == DC - 1:
                            nc.vector.tensor_add(
                                vaug[m][:, :, CO:CW],
                                vstate[0][:].rearrange("p (h d) -> p h d", h=NH),
                                vlev_t[m][:].rearrange("p (h d) -> p h d", h=NH),
                            )
                            nc.vector.memset(vaug[m][:, :, 0:CO], 0.0)
                            nc.vector.memset(vaug[m][:, :, 0:1], 1.0)
                    fillers.append(mmf)
                return fillers

            def run_fillers(fillers, k):
                for _ in range(min(k, len(fillers))):
                    fillers.pop(0)()

            last_act = {}

            def emit_attention(qb, hp, fillers=None, per_kc=3, hooks=None, sched=None):
                qs = slice(qb * NB, (qb + 1) * NB)
                fillers = fillers if fillers is not None else []
                hooks = hooks or {}
                cps = [
                    psum.tile([CW, NB], f32, name=f"cps{e}", tag="ctxps", bufs=2)
                    for e in range(2)
                ]
                # software pipeline per kc: scores(kc); exp(kc); PE filler
                # work (projections/outproj) in the exp-wait gap; ctx(kc-2)
                # (lag 2 so ctx never waits on the just-issued exp; probs
                # bufs=4 covers the extra in-flight tile)
                LAG = 2
                prs = {}
                for kc in range(KC + LAG):
                    if kc < KC:
                        sps = psum.tile([P, 2 * NB], f32, name="sps", tag="sps", bufs=2)
                        for e in range(2):
                            rows = slice(e * HD, (e + 1) * HD)
                            # head pair packed in PE row halves
                            nc.tensor.matmul(
                                sps[:, e * NB : (e + 1) * NB],
                                kT[hp][rows, kc * P : (kc + 1) * P],
                                qT[hp][rows, qs],
                                start=True,
                                stop=True,
                            )
                        pr = prp.tile([P, 2 * NB], bf16, name="pr", tag="pr")
                        ai = nc.scalar.activation(pr[:], sps[:], Exp, scale=1.0 / 8.0)
                        last_act["ai"] = ai
                        prs[kc] = pr
                        for h in hooks.get(kc, []):
                            h(ai)
                        run_fillers(fillers, sched[kc] if sched else per_kc)
                    if kc >= LAG:
                        pkc = kc - LAG
                        ppr = prs.pop(pkc)
                        for e in range(2):
                            nc.tensor.matmul(
                                cps[e][:],
                                vaug[pkc][:, 2 * hp + e, :],
                                ppr[:, e * NB : (e + 1) * NB],
                                start=(pkc == 0),
                                stop=(pkc == KC - 1),
                            )
                run_fillers(fillers, len(fillers))
                recbs = []
                for e in range(2):
                    rows = slice(e * HD, (e + 1) * HD)
                    # 1/denominator straight off PSUM row 0 — no staging
                    # copy or DMA.
                    recf = nrm.tile([1, NB], f32, name=f"recf{e}", tag=f"recf{e}", bufs=1)
                    nc.vector.reciprocal_approx_fast(recf[:], cps[e][0:1, :])
                    recb = nrm.tile([1, NB], bf16, name=f"recb{e}", tag=f"recb{e}", bufs=1)
                    nc.vector.tensor_copy(recb[:], recf[:])
                    recbs.append(recb)
                    # copy unnormalized ctx (normalized in place later)
                    nc.vector.tensor_copy(ctxT[hp][rows, qs], cps[e][CO:CW, :])
                return (qb, hp, recbs)

            def emit_norm_finish(pend):
                # Normalize a head pair (deferred into the next call's
                # filler stream): broadcast each head's reciprocal across
                # its 64 head-dim partitions via a K=1 matmul against the
                # partition-64 ones row.
                qb, hp, recbs = pend
                qs = slice(qb * NB, (qb + 1) * NB)
                for e in range(2):
                    rows = slice(e * HD, (e + 1) * HD)
                    bc = psum.tile([P, NB], f32, name="bc", tag="ps_proj", bufs=2)
                    nc.tensor.matmul(
                        bc[:], ones_t[:], recbs[e][:], start=True, stop=True
                    )
                    nc.vector.tensor_mul(
                        ctxT[hp][rows, qs], ctxT[hp][rows, qs], bc[0:HD, :]
                    )

            def outproj_fillers(qg, n):
                state = {}
                fillers = []
                for dc in range(MC):
                    def mmf(dc=dc):
                        if dc == 0:
                            state[0] = psum.tile(
                                [P, NB], f32, name="pso", tag="ps_proj", bufs=2
                            )
                        nc.tensor.matmul(
                            state[0][:],
                            ctxT[dc][:, qg * P : (qg + 1) * P],
                            wo_sb[:, dc, n * NB : (n + 1) * NB],
                            start=(dc == 0),
                            stop=(dc == MC - 1),
                        )
                        if dc == MC - 1:
                            ot = nrm.tile([P, NB], bf16, name="ot", tag="otile", bufs=4)
                            nc.vector.tensor_copy(ot[:], state[0][:])
                            nc.sync.dma_start(
                                out_d[qg * P : (qg + 1) * P, n * NB : (n + 1) * NB],
                                ot[:],
                            )
                    fillers.append(mmf)
                return fillers

            # ---- interleaved schedule ----
            # Projection-first: the projection phase is DMA-bound anyway, so
            # kT[0], qT[*][qb0] and the whole v projection run before
            # attention, with the PE trailing the input stream at full DMA
            # bandwidth and ScalarE idle. From call 1 on, every attention
            # call is exp-paced (~17.8us) and carries light PE fillers
            # (next kT chunk / next qT groups / previous query block's
            # output projection) in its exp-wait gaps.
            kt0 = kT_chunk_fillers(0)
            for f in kt0[0:DC]:  # n-block 0 (kin-n0 + wk-c0 only)
                f()
            for f in qT_group_fillers(0, 0):
                f()

            # Call 1 (qb0, hp0) carries every remaining projection as PE
            # fillers, in data-arrival order (engine queues are in-order at
            # runtime and the static scheduler does not model DMA latency,
            # so emission order must match the transfer stream). ScalarE
            # starts its exp stream ~15us in. Deadlines: kT[0] n-block j
            # fully emitted at least one step before scores(4j); vaug[m]
            # fully emitted by step m+LAG (the ctx emission point); the
            # lazy vt/kin/wk DMA emissions land on the sync queue between
            # the matching fillers.
            vch = [v_chunk_fillers(m) for m in range(KC)]
            c1 = []
            for m in range(1, MC):
                c1 += qT_group_fillers(m, 0)          # pos 0-23
            c1 += vch[0]                              # pos 24-31 (vt-c0 dma)
            c1 += kt0[DC : 2 * DC]                    # pos 32-39: kT0-n1
            c1.append(lambda: nc.sync.dma_start(
                kin[:, :, 2 * NB : 3 * NB], kt_v[:, :, 2 * NB : 3 * NB]))
            c1 += vch[1] + vch[2] + vch[3] + vch[4]   # pos 41-72 (vt-c1 @ 65)
            c1 += kt0[2 * DC : 3 * DC]                # pos 73-80: kT0-n2
            c1.append(lambda: nc.sync.dma_start(
                kin[:, :, 3 * NB : S], kt_v[:, :, 3 * NB : S]))
            c1 += vch[5] + vch[6] + vch[7]            # pos 82-105
            c1 += kt0[3 * DC :]                       # pos 106-113: kT0-n3
            c1 += vch[8] + vch[9] + vch[10] + vch[11] + vch[12]  # vt-c2/c3
            c1.append(lambda: nc.sync.dma_start(
                wk_sb[:, :, P:HH], wk_v[:, :, P:HH]))
            c1 += vch[13] + vch[14] + vch[15]
            c1 += kT_chunk_fillers(1)                 # call 2's kT chunk

            pend = None
            for qb in range(NQB):
                with nc.named_scope(f"qb{qb}"):
                    for hp in range(MC):
                        fillers = []
                        if qb == 0:
                            if hp == 0:
                                fillers += c1
                            if hp == 2:
                                # wo first used from qb1-hp1; gpsimd queue
                                # keeps it off the startup DMA window.
                                nc.gpsimd.dma_start(wo_sb[:], wo_v[:])
                            if 0 < hp < MC - 1:
                                fillers += kT_chunk_fillers(hp + 1)
                        else:
                            if hp == 0:
                                for m in range(1, MC):
                                    fillers += qT_group_fillers(m, qb)
                            else:
                                # previous qb's outproj: 8 groups over 3 calls
                                og = [(4 * (qb - 1) + g, n) for g in range(4) for n in range(2)]
                                take = {1: og[0:3], 2: og[3:6], 3: og[6:8]}[hp]
                                for qg, n in take:
                                    fillers += outproj_fillers(qg, n)
                        if hp == 1 and qb + 1 < NQB:
                            load_qin(qb + 1)
                        if hp == MC - 1 and qb + 1 < NQB:
                            fillers += qT_group_fillers(0, qb + 1)
                        # Normalize the previous call's head pair as this
                        # call's first filler: keeps the call-boundary PE
                        # chain down to ctx(15)->scores(0) so exp never
                        # waits on the normalize matmul.
                        if pend is not None:
                            prev = pend
                            fillers.insert(0, lambda prev=prev: emit_norm_finish(prev))
                        per_kc = (len(fillers) + KC - 1) // KC
                        pend = emit_attention(qb, hp, fillers, per_kc=max(per_kc, 1))

            def emit_outproj_tail(qg, n, upto=MC, tag="ctxps", use_scalar=False):
                # dc 0..upto-1 into a fresh psum group; rest + epilogue later.
                # Rides the attention's (now idle) PSUM slots; the final
                # normalize's bcast matmul keeps a free ps_proj slot.
                if tag in ("spsA", "spsB"):
                    # ride one half of an (idle) sps slot: two tail groups
                    # share each [128, 1024] scores slot
                    half = 0 if tag == "spsA" else 1
                    ps = psum.tile([P, 2 * NB], f32, name="pso", tag="sps", bufs=2)[
                        :, half * NB : (half + 1) * NB
                    ]
                else:
                    ps = psum.tile([P, NB], f32, name="pso", tag=tag, bufs=2)
                for dc in range(upto):
                    nc.tensor.matmul(
                        ps[:],
                        ctxT[dc][:, qg * P : (qg + 1) * P],
                        wo_sb[:, dc, n * NB : (n + 1) * NB],
                        start=(dc == 0),
                        stop=(dc == MC - 1),
                    )
                def finish():
                    for dc in range(upto, MC):
                        nc.tensor.matmul(
                            ps[:],
                            ctxT[dc][:, qg * P : (qg + 1) * P],
                            wo_sb[:, dc, n * NB : (n + 1) * NB],
                            start=False,
                            stop=(dc == MC - 1),
                        )
                    ot = nrm.tile([P, NB], bf16, name="ot2", tag="otile", bufs=4)
                    if use_scalar:
                        # ScalarE is idle after the final exp; the explicit
                        # dep keeps the scheduler from hoisting these casts
                        # into the exp stream (in-order queue = HOL risk).
                        ci = nc.scalar.copy(ot[:], ps[:])
                        _add_dep_helper(ci.ins, last_act["ai"].ins, sync=True, reason="tail")
                    else:
                        nc.vector.tensor_copy(ot[:], ps[:])
                    nc.sync.dma_start(
                        out_d[qg * P : (qg + 1) * P, n * NB : (n + 1) * NB], ot[:]
                    )
                return finish

            with nc.named_scope("outproj_tail"):
                # last qb's 8 outproj groups; hp 0..2's ctxT chunks are
                # normalized already, so dc 0..2 partials for 5 groups
                # (spread over the now-idle attention PSUM slots) keep the
                # PE at full clock (HAM) through the final normalize's DVE
                # chain; only the dc=3 matmul + epilogue remain per group.
                tail = [(4 * (NQB - 1) + g, n) for g in range(4) for n in range(2)]
                slots = ["ctxps", "ctxps", "spsA", "spsA", "spsB", "spsB", "ps_proj"]
                fins = [
                    emit_outproj_tail(qg, n, upto=MC - 1, tag=tag, use_scalar=(i % 2 == 0))
                    for i, ((qg, n), tag) in enumerate(zip(tail[:7], slots))
                ]
                emit_norm_finish(pend)
                for f in fins:
                    f()
                emit_outproj_tail(*tail[7], use_scalar=True)()

    nc.compile()
    return nc


def get_nc():
    if "nc" not in _BUILD_CACHE:
        _BUILD_CACHE["nc"] = _build_nc()
    return _BUILD_CACHE["nc"]


def make_in_maps(inputs):
    bf16 = ml_dtypes.bfloat16
    f32 = np.float32
    Q = np.asarray(inputs["Q"], f32)
    Q_lev = np.asarray(inputs["Q_lev"], f32)
    K = np.asarray(inputs["K"], f32)
    K_lev = np.asarray(inputs["K_lev"], f32)
    V = np.asarray(inputs["V"], f32)
    V_lev = np.asarray(inputs["V_lev"], f32)
    bq = np.asarray(inputs["bq"], f32)
    bk = np.asarray(inputs["bk"], f32)
    bv = np.asarray(inputs["bv"], f32)
    Wq = np.asarray(inputs["Wq"], f32)
    Wk = np.asarray(inputs["Wk"], f32)
    Wv = np.asarray(inputs["Wv"], f32)
    Wo = np.asarray(inputs["Wo"], f32)

    per_batch = []
    for b in range(B):
        per_batch.append(
            {
                "qt": np.ascontiguousarray(Q[b].T.astype(bf16)),
                "kt": np.ascontiguousarray(K[b].T.astype(bf16)),
                "vt": np.ascontiguousarray(V[b].T.astype(bf16)),
            }
        )
    qlevT = [np.ascontiguousarray((Q_lev[b] + bq).T).astype(bf16) for b in range(B)]
    klevT = [np.ascontiguousarray((K_lev[b] + bk).T).astype(bf16) for b in range(B)]
    vlev = [np.ascontiguousarray(V_lev[b] + bv).astype(bf16) for b in range(B)]

    in_maps = []
    for c in range(N_CORES):
        b, hh = divmod(c, 2)
        fs = slice(hh * HH, (hh + 1) * HH)
        in_maps.append(
            {
                **per_batch[b],
                "qlev": np.ascontiguousarray(qlevT[b][fs]),
                "klev": np.ascontiguousarray(klevT[b][fs]),
                "vlev": np.ascontiguousarray(vlev[b][:, fs]),
                "wq": np.ascontiguousarray(Wq[:, fs].astype(bf16)),
                "wk": np.ascontiguousarray(Wk[:, fs].astype(bf16)),
                "wv": np.ascontiguousarray(Wv[:, fs].astype(bf16)),
                "wo": np.ascontiguousarray(Wo[fs, :].astype(bf16)),
            }
        )
    return in_maps


def combine_outputs(results, inputs):
    bo = np.asarray(inputs["bo"], np.float32)
    out = np.empty((B, S, D), np.float32)
    for b in range(B):
        out[b] = (
            results[2 * b]["out"].astype(np.float32)
            + results[2 * b + 1]["out"].astype(np.float32)
            + bo
        )
    return out


def run_on_cores(inputs, trace=False):
    """Run the SPMD kernel; returns (full_output, BassKernelResults)."""
    from concourse.bass_utils import run_bass_kernel_spmd

    nc = get_nc()
    in_maps = make_in_maps(inputs)
    res = run_bass_kernel_spmd(nc, in_maps, core_ids=list(range(N_CORES)), trace=trace)
    return combine_outputs(res.results, inputs), res


def kernel(**inputs):
    out, _ = run_on_cores(inputs, trace=False)
    return out


if __name__ == "__main__":
    nc = get_nc()
    print("built + compiled OK")
